# revision 15
# baseline (speedup 1.0000x reference)
"""ChannelDeconv (training-mode forward, C == block == 64) on 8 TRN2 NeuronCores.

Reference math:
    x: (32, 64, 128, 128) f32, NCHW
    x1    = x.transpose(1,0,2,3).reshape(64, N*H*W)        # [B, L], L = 524288
    x1_s  = x1[:, ::9]                                     # 58255 sampled cols
    mean  = x1_s.mean(-1); cov = x1_s @ x1_s.T / n_s + 0.01*I
    D     = newton_schulz_isqrt(cov, 5)
    y     = D @ (x1 - mean)  -> back to NCHW

Sharding: data-parallel over N. Core k owns images [4k, 4k+4) stacked as
[128, 32768] bf16 (partitions 0:64 = channels of images 0,1; 64:128 =
channels of images 2,3).

Statistics are REPLICATED, not all-reduced: a prior version computed
per-core partial Grams and AllReduce'd the [65,65] stats, but the
collective cost ~69us on-core (44us inter-core arrival-skew barrier +
14us AllReduce + trigger overhead) out of a 137us kernel. Instead the
host gathers the full global stride-9 sample set once and EVERY core
redundantly computes the full Gram from it:
  - xst: [128, 228*129] fp8e4m3. 456 chunks of 128 samples x 64 channels,
    packed two chunks per 129-col "pair": [A(64ch) | B(64ch) | ones].
    Each pair is ONE 128-col-weight matmul accumulating A^T A into PSUM
    quadrant [0:64,0:64], B^T B into [64:128,64:128], and per-channel
    column sums into col 128 (the ones column). ~59ns/pair on the PE.
  - fp8 sample rounding (~3.6%/elem) perturbs cov entries by ~2e-4 <<
    the reference's own sampling noise; stats math stays f32. Measured
    rel err vs the f32 reference: 2.7e-3 (budget 2e-2).
  - quadrant fold (partitions 64:128 -> 0:64): tiny SBUF->SBUF DMA
    remap + DVE adds (cross-partition adds are not a DVE primitive).
Whitening y = [[D,0],[0,D]] @ S - D@mean runs in bf16 (128-col FWL
weights, f32 PSUM, bias applied at eviction split across DVE/ACT);
output is bf16 in 16 contiguous 512KB stores alternating the two HWDGE
rings; the host upcasts to f32.

HW: 82.5us/core (baseline with AllReduce: 134-137us). Known-but-
unlandable: moving all 16 store triggers to the sync ring, 8-piece xst,
f32r Newton-Schulz, and PE pre-warm each hit NRT_EXEC_UNIT_UNRECOVERABLE
or regressed when combined; see session notes.
"""

import importlib.util
import os
import sys

if importlib.util.find_spec("concourse") is None:
    for _p in ("/opt/trn_rl_repo", os.path.expanduser("~/.axon_site/_ro/trn_rl_repo")):
        if os.path.isdir(_p) and _p not in sys.path:
            sys.path.insert(0, _p)

import numpy as np

N, C, H, W = 32, 64, 128, 128
HW = H * W               # 16384
B = 64                   # whitening block / channel count
STRIDE2 = 9              # sampling stride**2
EPS = 0.01
N_ITER = 5
CORES = 8
NL = N // CORES          # images per core = 4
WIDE = 2 * HW            # stacked free dim = 32768
NS_TOT = (N * HW + STRIDE2 - 1) // STRIDE2   # 58255 global samples
NPAIR = 228              # pairs of 128-sample chunks (228*256 = 58368 slots)
PAIRW = 2 * B + 1        # 129: [A 64ch | B 64ch | ones]
XST_W = NPAIR * PAIRW    # 29412 fp8 bytes per partition

_cached = {}


def _build_nc():
    import concourse.mybir as mybir
    import concourse.tile as tile
    from concourse import bacc

    f32 = mybir.dt.float32
    bf16 = mybir.dt.bfloat16
    f8 = mybir.dt.float8e4
    nc = bacc.Bacc(None, num_devices=CORES)

    xp = nc.declare_dram_parameter("xp", [128, WIDE], bf16, isOutput=False)
    xst = nc.declare_dram_parameter("xst", [128, XST_W], f8, isOutput=False)
    eye_in = nc.declare_dram_parameter("eye", [B, B], f32, isOutput=False)
    NBLK = 512                      # one PSUM bank of f32
    GRP = 4                         # whitening blocks per output DMA
    NJO = WIDE // (NBLK * GRP)      # 16 output stores of 512KB
    out_ext = nc.declare_dram_parameter("out", [NJO, 128, NBLK * GRP], bf16,
                                        isOutput=True)

    with tile.TileContext(nc) as tc:
        with (
            tc.tile_pool(name="big", bufs=1) as big,
            tc.tile_pool(name="stage", bufs=1) as stage,
            tc.tile_pool(name="smalls", bufs=1) as smalls,
            tc.tile_pool(name="nsp", bufs=2) as nsp,
            tc.tile_pool(name="psg", bufs=1, space="PSUM") as psg,
            tc.tile_pool(name="pss", bufs=2, space="PSUM") as pss,
            tc.tile_pool(name="psw", bufs=5, space="PSUM") as psw,
            tc.tile_pool(name="outs", bufs=4) as outs,
        ):
            # ---- input DMAs -------------------------------------------------
            xst_sb = stage.tile([128, XST_W], f8)
            NXP = 4
            PPP = NPAIR // NXP          # 57 pairs per piece
            for i in range(NXP):
                nc.sync.dma_start(
                    out=xst_sb[:, i * PPP * PAIRW:(i + 1) * PPP * PAIRW],
                    in_=xst[:, i * PPP * PAIRW:(i + 1) * PPP * PAIRW],
                )
            eye_sb = smalls.tile([B, B], f32)
            nc.scalar.dma_start(out=eye_sb[:, :], in_=eye_in[:, :])
            S = big.tile([128, WIDE], bf16)
            NSPLIT = 8
            csz = WIDE // NSPLIT
            for i in range(NSPLIT):
                nc.sync.dma_start(
                    out=S[:, i * csz:(i + 1) * csz],
                    in_=xp[:, i * csz:(i + 1) * csz],
                )

            # ---- replicated global Gram (+ column sums via ones column) -----
            g_ps = psg.tile([128, PAIRW], f32, tag="g")
            for p in range(NPAIR):
                nc.tensor.matmul(
                    g_ps[:, :],
                    lhsT=xst_sb[:, p * PAIRW:p * PAIRW + 2 * B],
                    rhs=xst_sb[:, p * PAIRW:(p + 1) * PAIRW],
                    start=(p == 0), stop=(p == NPAIR - 1),
                )
            g_sb = smalls.tile([128, PAIRW], f32)
            nc.vector.tensor_copy(out=g_sb[:, :], in_=g_ps[:, :])
            # fold the DIAGONAL quadrants via DMA partition remap
            g_hi = smalls.tile([B, B + 1], f32)
            nc.scalar.dma_start(out=g_hi[:, :], in_=g_sb[B:128, B:PAIRW])
            gsum = smalls.tile([B, B + 1], f32)
            nc.vector.tensor_add(out=gsum[:, 0:B], in0=g_sb[0:B, 0:B],
                                 in1=g_hi[:, 0:B])
            nc.vector.tensor_add(out=gsum[:, B:B + 1],
                                 in0=g_sb[0:B, 2 * B:2 * B + 1],
                                 in1=g_hi[:, B:B + 1])
            # G/n and mean = colsum/n in one scale over [64, 65]
            ms = smalls.tile([B, B + 1], f32)
            nc.vector.tensor_scalar_mul(out=ms[:, :], in0=gsum[:, :],
                                        scalar1=1.0 / NS_TOT)
            mean_ap = ms[:, B:B + 1]

            # ---- cov = G/n + eps*I ------------------------------------------
            eps_eye = smalls.tile([B, B], f32)
            nc.vector.tensor_scalar_mul(out=eps_eye[:, :], in0=eye_sb[:, :],
                                        scalar1=EPS)
            eye3 = smalls.tile([B, B], f32)
            nc.vector.tensor_scalar_mul(out=eye3[:, :], in0=eye_sb[:, :],
                                        scalar1=3.0)
            ones64 = smalls.tile([B, B], f32)
            nc.vector.memset(ones64[:, :], 1.0)

            cov = smalls.tile([B, B], f32)
            sq = smalls.tile([B, B], f32)
            rsum = smalls.tile([B, 1], f32)
            nc.vector.tensor_add(out=cov[:, :], in0=ms[:, 0:B],
                                 in1=eps_eye[:, :])
            nc.vector.tensor_mul(out=sq[:, :], in0=cov[:, :], in1=cov[:, :])
            nc.vector.reduce_sum(out=rsum[:, :], in_=sq[:, :],
                                 axis=mybir.AxisListType.X)
            nsq_ps = pss.tile([B, 1], f32, tag="pss")
            nc.tensor.matmul(nsq_ps[:, :], lhsT=ones64[:, :], rhs=rsum[:, :],
                             start=True, stop=True)
            normA = smalls.tile([B, 1], f32)
            nc.scalar.activation(out=normA[:, :], in_=nsq_ps[:, :],
                                 func=mybir.ActivationFunctionType.Sqrt)
            rnorm = smalls.tile([B, 1], f32)
            nc.vector.reciprocal(out=rnorm[:, :], in_=normA[:, :])
            rqnorm = smalls.tile([B, 1], f32)
            nc.scalar.activation(out=rqnorm[:, :], in_=rnorm[:, :],
                                 func=mybir.ActivationFunctionType.Sqrt)
            rqnorm_nh = smalls.tile([B, 1], f32)
            nc.scalar.activation(out=rqnorm_nh[:, :], in_=rqnorm[:, :],
                                 func=mybir.ActivationFunctionType.Copy,
                                 scale=0.5)

            # ---- Newton-Schulz ----------------------------------------------
            U = nsp.tile([B, 2 * B], f32, tag="U", name="U0")
            nc.vector.tensor_scalar_mul(out=U[:, 0:B], in0=cov[:, :],
                                        scalar1=rnorm[:, :])
            nc.vector.tensor_copy(out=U[:, B:2 * B], in_=eye_sb[:, :])
            for it in range(N_ITER):
                last = it == N_ITER - 1
                T = nsp.tile([B, B], f32, tag="T", name=f"T{it}")
                if it == 0:
                    nc.vector.tensor_sub(out=T[:, :], in0=eye3[:, :], in1=U[:, 0:B])
                else:
                    zy_ps = pss.tile([B, B], f32, tag="pss", name=f"zy{it}")
                    nc.tensor.matmul(zy_ps[:, :], lhsT=U[:, B:2 * B],
                                     rhs=U[:, 0:B], start=True, stop=True)
                    nc.vector.tensor_sub(out=T[:, :], in0=eye3[:, :], in1=zy_ps[:, :])
                rhs = U[:, B:2 * B] if last else U[:, :]
                un_ps = pss.tile([B, B if last else 2 * B], f32, tag="pss",
                                 name=f"un{it}")
                nc.tensor.matmul(un_ps[:, :], lhsT=T[:, :], rhs=rhs,
                                 start=True, stop=True)
                if last:
                    break
                Un = nsp.tile([B, 2 * B], f32, tag="U", name=f"U{it + 1}")
                nc.vector.tensor_scalar_mul(out=Un[:, :], in0=un_ps[:, :],
                                            scalar1=0.5)
                U = Un

            deconv = smalls.tile([B, B], f32)
            nc.vector.tensor_scalar_mul(out=deconv[:, :], in0=un_ps[:, :],
                                        scalar1=rqnorm_nh[:, :])

            # ---- block-diagonal [[D,0],[0,D]] in bf16 -----------------------
            dblk_ps = pss.tile([128, 128], f32, tag="pss")
            nc.tensor.matmul(dblk_ps[0:B, 0:B], lhsT=deconv[:, :], rhs=eye_sb[:, :],
                             start=True, stop=True)
            nc.tensor.matmul(dblk_ps[B:128, B:128], lhsT=deconv[:, :],
                             rhs=eye_sb[:, :], start=True, stop=True)
            dblk = smalls.tile([128, 128], bf16)
            zeros128 = smalls.tile([128, B], f32)
            nc.vector.memset(zeros128[:, :], 0.0)
            nc.vector.tensor_copy(out=dblk[0:B, B:128], in_=zeros128[0:B, :])
            nc.vector.tensor_copy(out=dblk[B:128, 0:B], in_=zeros128[B:128, :])
            nc.vector.tensor_copy(out=dblk[0:B, 0:B], in_=dblk_ps[0:B, 0:B])
            nc.vector.tensor_copy(out=dblk[B:128, B:128], in_=dblk_ps[B:128, B:128])

            # ---- stacked bias dm2 = [D@mean; D@mean] ------------------------
            dm2_ps = pss.tile([128, 1], f32, tag="pss")
            nc.tensor.matmul(dm2_ps[0:B, :], lhsT=deconv[:, :], rhs=mean_ap,
                             start=True, stop=True)
            nc.tensor.matmul(dm2_ps[B:128, :], lhsT=deconv[:, :],
                             rhs=mean_ap, start=True, stop=True)
            negdm2 = smalls.tile([128, 1], f32)
            nc.vector.tensor_scalar_mul(out=negdm2[:, :], in0=dm2_ps[:, :],
                                        scalar1=-1.0)

            # ---- whitening: y = Dblk @ S - dm2 ------------------------------
            for jo in range(NJO):
                y_sb = outs.tile([128, NBLK * GRP], bf16, tag="y", name=f"y{jo}")
                for ji in range(GRP):
                    j = jo * GRP + ji
                    w_ps = psw.tile([128, NBLK], f32, tag="w", name=f"w{j}")
                    nc.tensor.matmul(
                        w_ps[:, :], lhsT=dblk[:, :],
                        rhs=S[:, j * NBLK:(j + 1) * NBLK],
                        start=True, stop=True,
                    )
                    HB = NBLK // 2
                    nc.vector.tensor_scalar_add(
                        out=y_sb[:, ji * NBLK:ji * NBLK + HB],
                        in0=w_ps[:, 0:HB], scalar1=negdm2[:, :],
                    )
                    nc.scalar.activation(
                        out=y_sb[:, ji * NBLK + HB:(ji + 1) * NBLK],
                        in_=w_ps[:, HB:NBLK],
                        func=mybir.ActivationFunctionType.Identity,
                        bias=negdm2[:, :], scale=1.0,
                    )
                eng = nc.scalar if jo % 2 == 0 else nc.sync
                eng.dma_start(out=out_ext[jo, :, :], in_=y_sb[:, :])

    nc.finalize()
    return nc


def _shard_inputs(x):
    import ml_dtypes

    bf16 = ml_dtypes.bfloat16
    f8 = ml_dtypes.float8_e4m3

    x = np.ascontiguousarray(x, dtype=np.float32)
    xr = x.reshape(N, C, HW)

    ls = np.arange(0, N * HW, STRIDE2, dtype=np.int64)
    ns_idx = ls // HW
    hw_idx = ls % HW
    xs_all = xr[ns_idx, :, hw_idx]

    xs_pad = np.zeros((NPAIR * 256, B), dtype=np.float32)
    xs_pad[:NS_TOT] = xs_all
    xs3 = xs_pad.reshape(NPAIR, 2, 128, B)
    xst_np = np.empty((128, NPAIR, PAIRW), dtype=np.float32)
    xst_np[:, :, 0:B] = xs3[:, 0].transpose(1, 0, 2)
    xst_np[:, :, B:2 * B] = xs3[:, 1].transpose(1, 0, 2)
    xst_np[:, :, 2 * B] = 1.0
    xst_f8 = np.ascontiguousarray(
        xst_np.reshape(128, XST_W)).astype(f8)

    eye = np.eye(B, dtype=np.float32)
    in_maps = []
    for k in range(CORES):
        x4 = x[NL * k:NL * (k + 1)].reshape(2, 2, C, HW)
        xp = np.ascontiguousarray(
            x4.transpose(0, 2, 1, 3).reshape(128, WIDE)).astype(bf16)
        in_maps.append({"xp": xp, "xst": xst_f8, "eye": eye})
    return in_maps


def _unshard_output(results):
    y = np.empty((N, C, H, W), dtype=np.float32)
    for k in range(CORES):
        o = np.asarray(results[k]["out"]).astype(np.float32)
        o = o.reshape(16, 128, 2048).transpose(1, 0, 2).reshape(128, WIDE)
        o = o.reshape(2, C, 2, HW)
        y[NL * k:NL * (k + 1)] = (
            o.transpose(0, 2, 1, 3).reshape(NL, C, H, W))
    return y


def kernel(x):
    from concourse.bass_utils import run_bass_kernel_spmd

    if "nc" not in _cached:
        _cached["nc"] = _build_nc()
    nc = _cached["nc"]

    in_maps = _shard_inputs(np.asarray(x))
    res = run_bass_kernel_spmd(nc, in_maps, core_ids=list(range(CORES)))
    _cached["last_results"] = res
    return _unshard_output(res.results)


# revision 16
# speedup vs baseline: 1.0011x; 1.0011x over previous
"""ChannelDeconv (training-mode forward, C == block == 64) on 8 TRN2 NeuronCores.

Reference math:
    x: (32, 64, 128, 128) f32, NCHW
    x1    = x.transpose(1,0,2,3).reshape(64, N*H*W)        # [B, L], L = 524288
    x1_s  = x1[:, ::9]                                     # 58255 sampled cols
    mean  = x1_s.mean(-1); cov = x1_s @ x1_s.T / n_s + 0.01*I
    D     = newton_schulz_isqrt(cov, 5)
    y     = D @ (x1 - mean)  -> back to NCHW

Sharding: data-parallel over N. Core k owns images [4k, 4k+4) stacked as
[128, 32768] bf16 (partitions 0:64 = channels of images 0,1; 64:128 =
channels of images 2,3).

Statistics are REPLICATED, not all-reduced: a prior version computed
per-core partial Grams and AllReduce'd the [65,65] stats, but the
collective cost ~69us on-core (44us inter-core arrival-skew barrier +
14us AllReduce + trigger overhead) out of a 137us kernel. Instead the
host gathers the full global stride-9 sample set once and EVERY core
redundantly computes the full Gram from it:
  - xst: [128, 228*129] fp8e4m3. 456 chunks of 128 samples x 64 channels,
    packed two chunks per 129-col "pair": [A(64ch) | B(64ch) | ones].
    Each pair is ONE 128-col-weight matmul accumulating A^T A into PSUM
    quadrant [0:64,0:64], B^T B into [64:128,64:128], and per-channel
    column sums into col 128 (the ones column). ~59ns/pair on the PE.
  - fp8 sample rounding (~3.6%/elem) perturbs cov entries by ~2e-4 <<
    the reference's own sampling noise; stats math stays f32. Measured
    rel err vs the f32 reference: 2.7e-3 (budget 2e-2).
  - quadrant fold (partitions 64:128 -> 0:64): tiny SBUF->SBUF DMA
    remap + DVE adds (cross-partition adds are not a DVE primitive).
Whitening y = [[D,0],[0,D]] @ S - D@mean runs in bf16 (128-col FWL
weights, f32 PSUM, bias applied at eviction split across DVE/ACT);
output is bf16 in 16 contiguous 512KB stores alternating the two HWDGE
rings; the host upcasts to f32.

HW: 82.5us/core (baseline with AllReduce: 134-137us). Known-but-
unlandable: moving all 16 store triggers to the sync ring, 8-piece xst,
f32r Newton-Schulz, and PE pre-warm each hit NRT_EXEC_UNIT_UNRECOVERABLE
or regressed when combined; see session notes.
"""

import importlib.util
import os
import sys

if importlib.util.find_spec("concourse") is None:
    for _p in ("/opt/trn_rl_repo", os.path.expanduser("~/.axon_site/_ro/trn_rl_repo")):
        if os.path.isdir(_p) and _p not in sys.path:
            sys.path.insert(0, _p)

import numpy as np

N, C, H, W = 32, 64, 128, 128
HW = H * W               # 16384
B = 64                   # whitening block / channel count
STRIDE2 = 9              # sampling stride**2
EPS = 0.01
N_ITER = 5
CORES = 8
NL = N // CORES          # images per core = 4
WIDE = 2 * HW            # stacked free dim = 32768
NS_TOT = (N * HW + STRIDE2 - 1) // STRIDE2   # 58255 global samples
NPAIR = 228              # pairs of 128-sample chunks (228*256 = 58368 slots)
PAIRW = 2 * B + 1        # 129: [A 64ch | B 64ch | ones]
XST_W = NPAIR * PAIRW    # 29412 fp8 bytes per partition

_cached = {}


def _build_nc():
    import concourse.mybir as mybir
    import concourse.tile as tile
    from concourse import bacc

    f32 = mybir.dt.float32
    bf16 = mybir.dt.bfloat16
    f8 = mybir.dt.float8e4
    nc = bacc.Bacc(None, num_devices=CORES)

    xp = nc.declare_dram_parameter("xp", [128, WIDE], bf16, isOutput=False)
    xst = nc.declare_dram_parameter("xst", [128, XST_W], f8, isOutput=False)
    eye_in = nc.declare_dram_parameter("eye", [B, B], f32, isOutput=False)
    NBLK = 512                      # one PSUM bank of f32
    GRP = 4                         # whitening blocks per output DMA
    NJO = WIDE // (NBLK * GRP)      # 16 output stores of 512KB
    out_ext = nc.declare_dram_parameter("out", [NJO, 128, NBLK * GRP], bf16,
                                        isOutput=True)

    with tile.TileContext(nc) as tc:
        with (
            tc.tile_pool(name="big", bufs=1) as big,
            tc.tile_pool(name="stage", bufs=1) as stage,
            tc.tile_pool(name="smalls", bufs=1) as smalls,
            tc.tile_pool(name="nsp", bufs=2) as nsp,
            tc.tile_pool(name="psg", bufs=1, space="PSUM") as psg,
            tc.tile_pool(name="pss", bufs=2, space="PSUM") as pss,
            tc.tile_pool(name="psw", bufs=5, space="PSUM") as psw,
            tc.tile_pool(name="outs", bufs=4) as outs,
        ):
            # ---- input DMAs -------------------------------------------------
            xst_sb = stage.tile([128, XST_W], f8)
            NXP = 4
            PPP = NPAIR // NXP          # 57 pairs per piece
            for i in range(NXP):
                nc.sync.dma_start(
                    out=xst_sb[:, i * PPP * PAIRW:(i + 1) * PPP * PAIRW],
                    in_=xst[:, i * PPP * PAIRW:(i + 1) * PPP * PAIRW],
                )
            eye_sb = smalls.tile([B, B], f32)
            nc.scalar.dma_start(out=eye_sb[:, :], in_=eye_in[:, :])
            S = big.tile([128, WIDE], bf16)
            NSPLIT = 8
            csz = WIDE // NSPLIT
            for i in range(NSPLIT):
                nc.sync.dma_start(
                    out=S[:, i * csz:(i + 1) * csz],
                    in_=xp[:, i * csz:(i + 1) * csz],
                )

            # ---- constants (issued before the Gram so the DVE queue has
            # them done long before the post-Gram critical chain) ----------
            eps_eye = smalls.tile([B, B], f32)
            nc.vector.tensor_scalar_mul(out=eps_eye[:, :], in0=eye_sb[:, :],
                                        scalar1=EPS)
            eye3 = smalls.tile([B, B], f32)
            nc.vector.tensor_scalar_mul(out=eye3[:, :], in0=eye_sb[:, :],
                                        scalar1=3.0)
            ones64 = smalls.tile([B, B], f32)
            nc.vector.memset(ones64[:, :], 1.0)
            zeros128 = smalls.tile([128, B], f32)
            nc.vector.memset(zeros128[:, :], 0.0)
            dblk = smalls.tile([128, 128], bf16)
            nc.vector.tensor_copy(out=dblk[0:B, B:128], in_=zeros128[0:B, :])
            nc.vector.tensor_copy(out=dblk[B:128, 0:B], in_=zeros128[B:128, :])

            # ---- replicated global Gram (+ column sums via ones column) -----
            g_ps = psg.tile([128, PAIRW], f32, tag="g")
            for p in range(NPAIR):
                nc.tensor.matmul(
                    g_ps[:, :],
                    lhsT=xst_sb[:, p * PAIRW:p * PAIRW + 2 * B],
                    rhs=xst_sb[:, p * PAIRW:(p + 1) * PAIRW],
                    start=(p == 0), stop=(p == NPAIR - 1),
                )
            g_sb = smalls.tile([128, PAIRW], f32)
            nc.vector.tensor_copy(out=g_sb[:, :], in_=g_ps[:, :])
            # fold the DIAGONAL quadrants via DMA partition remap. SWDGE
            # (gpsimd): on the scalar HWDGE ring this 16.6KB remap queued
            # behind the S input stream and stalled the deconv chain ~7us.
            g_hi = smalls.tile([B, B + 1], f32)
            nc.gpsimd.dma_start(out=g_hi[:, :], in_=g_sb[B:128, B:PAIRW])
            gsum = smalls.tile([B, B + 1], f32)
            nc.vector.tensor_add(out=gsum[:, 0:B], in0=g_sb[0:B, 0:B],
                                 in1=g_hi[:, 0:B])
            nc.vector.tensor_add(out=gsum[:, B:B + 1],
                                 in0=g_sb[0:B, 2 * B:2 * B + 1],
                                 in1=g_hi[:, B:B + 1])
            # G/n and mean = colsum/n in one scale over [64, 65]
            ms = smalls.tile([B, B + 1], f32)
            nc.vector.tensor_scalar_mul(out=ms[:, :], in0=gsum[:, :],
                                        scalar1=1.0 / NS_TOT)
            mean_ap = ms[:, B:B + 1]

            # ---- cov = G/n + eps*I ------------------------------------------
            cov = smalls.tile([B, B], f32)
            sq = smalls.tile([B, B], f32)
            rsum = smalls.tile([B, 1], f32)
            nc.vector.tensor_add(out=cov[:, :], in0=ms[:, 0:B],
                                 in1=eps_eye[:, :])
            nc.vector.tensor_mul(out=sq[:, :], in0=cov[:, :], in1=cov[:, :])
            nc.vector.reduce_sum(out=rsum[:, :], in_=sq[:, :],
                                 axis=mybir.AxisListType.X)
            nsq_ps = pss.tile([B, 1], f32, tag="pss")
            nc.tensor.matmul(nsq_ps[:, :], lhsT=ones64[:, :], rhs=rsum[:, :],
                             start=True, stop=True)
            normA = smalls.tile([B, 1], f32)
            nc.scalar.activation(out=normA[:, :], in_=nsq_ps[:, :],
                                 func=mybir.ActivationFunctionType.Sqrt)
            rnorm = smalls.tile([B, 1], f32)
            nc.vector.reciprocal(out=rnorm[:, :], in_=normA[:, :])
            rqnorm = smalls.tile([B, 1], f32)
            nc.scalar.activation(out=rqnorm[:, :], in_=rnorm[:, :],
                                 func=mybir.ActivationFunctionType.Sqrt)
            rqnorm_nh = smalls.tile([B, 1], f32)
            nc.scalar.activation(out=rqnorm_nh[:, :], in_=rqnorm[:, :],
                                 func=mybir.ActivationFunctionType.Copy,
                                 scale=0.5)

            # ---- Newton-Schulz ----------------------------------------------
            U = nsp.tile([B, 2 * B], f32, tag="U", name="U0")
            nc.vector.tensor_scalar_mul(out=U[:, 0:B], in0=cov[:, :],
                                        scalar1=rnorm[:, :])
            nc.vector.tensor_copy(out=U[:, B:2 * B], in_=eye_sb[:, :])
            for it in range(N_ITER):
                last = it == N_ITER - 1
                T = nsp.tile([B, B], f32, tag="T", name=f"T{it}")
                if it == 0:
                    nc.vector.tensor_sub(out=T[:, :], in0=eye3[:, :], in1=U[:, 0:B])
                else:
                    zy_ps = pss.tile([B, B], f32, tag="pss", name=f"zy{it}")
                    nc.tensor.matmul(zy_ps[:, :], lhsT=U[:, B:2 * B],
                                     rhs=U[:, 0:B], start=True, stop=True)
                    nc.vector.tensor_sub(out=T[:, :], in0=eye3[:, :], in1=zy_ps[:, :])
                rhs = U[:, B:2 * B] if last else U[:, :]
                un_ps = pss.tile([B, B if last else 2 * B], f32, tag="pss",
                                 name=f"un{it}")
                nc.tensor.matmul(un_ps[:, :], lhsT=T[:, :], rhs=rhs,
                                 start=True, stop=True)
                if last:
                    break
                Un = nsp.tile([B, 2 * B], f32, tag="U", name=f"U{it + 1}")
                nc.vector.tensor_scalar_mul(out=Un[:, :], in0=un_ps[:, :],
                                            scalar1=0.5)
                U = Un

            deconv = smalls.tile([B, B], f32)
            nc.vector.tensor_scalar_mul(out=deconv[:, :], in0=un_ps[:, :],
                                        scalar1=rqnorm_nh[:, :])

            # ---- block-diagonal [[D,0],[0,D]] in bf16 -----------------------
            dblk_ps = pss.tile([128, 128], f32, tag="pss")
            nc.tensor.matmul(dblk_ps[0:B, 0:B], lhsT=deconv[:, :], rhs=eye_sb[:, :],
                             start=True, stop=True)
            nc.tensor.matmul(dblk_ps[B:128, B:128], lhsT=deconv[:, :],
                             rhs=eye_sb[:, :], start=True, stop=True)
            nc.vector.tensor_copy(out=dblk[0:B, 0:B], in_=dblk_ps[0:B, 0:B])
            nc.vector.tensor_copy(out=dblk[B:128, B:128], in_=dblk_ps[B:128, B:128])

            # ---- stacked bias dm2 = [D@mean; D@mean] ------------------------
            dm2_ps = pss.tile([128, 1], f32, tag="pss")
            nc.tensor.matmul(dm2_ps[0:B, :], lhsT=deconv[:, :], rhs=mean_ap,
                             start=True, stop=True)
            nc.tensor.matmul(dm2_ps[B:128, :], lhsT=deconv[:, :],
                             rhs=mean_ap, start=True, stop=True)
            negdm2 = smalls.tile([128, 1], f32)
            nc.vector.tensor_scalar_mul(out=negdm2[:, :], in0=dm2_ps[:, :],
                                        scalar1=-1.0)

            # ---- whitening: y = Dblk @ S - dm2 ------------------------------
            for jo in range(NJO):
                y_sb = outs.tile([128, NBLK * GRP], bf16, tag="y", name=f"y{jo}")
                for ji in range(GRP):
                    j = jo * GRP + ji
                    w_ps = psw.tile([128, NBLK], f32, tag="w", name=f"w{j}")
                    nc.tensor.matmul(
                        w_ps[:, :], lhsT=dblk[:, :],
                        rhs=S[:, j * NBLK:(j + 1) * NBLK],
                        start=True, stop=True,
                    )
                    # DVE takes 320 of 512 cols: the scalar engine also
                    # issues store triggers and paced the store phase
                    HB = 320
                    nc.vector.tensor_scalar_add(
                        out=y_sb[:, ji * NBLK:ji * NBLK + HB],
                        in0=w_ps[:, 0:HB], scalar1=negdm2[:, :],
                    )
                    nc.scalar.activation(
                        out=y_sb[:, ji * NBLK + HB:(ji + 1) * NBLK],
                        in_=w_ps[:, HB:NBLK],
                        func=mybir.ActivationFunctionType.Identity,
                        bias=negdm2[:, :], scale=1.0,
                    )
                eng = nc.scalar if jo % 2 == 0 else nc.sync
                eng.dma_start(out=out_ext[jo, :, :], in_=y_sb[:, :])

    nc.finalize()
    return nc


def _shard_inputs(x):
    import ml_dtypes

    bf16 = ml_dtypes.bfloat16
    f8 = ml_dtypes.float8_e4m3

    x = np.ascontiguousarray(x, dtype=np.float32)
    xr = x.reshape(N, C, HW)

    ls = np.arange(0, N * HW, STRIDE2, dtype=np.int64)
    ns_idx = ls // HW
    hw_idx = ls % HW
    xs_all = xr[ns_idx, :, hw_idx]

    xs_pad = np.zeros((NPAIR * 256, B), dtype=np.float32)
    xs_pad[:NS_TOT] = xs_all
    xs3 = xs_pad.reshape(NPAIR, 2, 128, B)
    xst_np = np.empty((128, NPAIR, PAIRW), dtype=np.float32)
    xst_np[:, :, 0:B] = xs3[:, 0].transpose(1, 0, 2)
    xst_np[:, :, B:2 * B] = xs3[:, 1].transpose(1, 0, 2)
    xst_np[:, :, 2 * B] = 1.0
    xst_f8 = np.ascontiguousarray(
        xst_np.reshape(128, XST_W)).astype(f8)

    eye = np.eye(B, dtype=np.float32)
    in_maps = []
    for k in range(CORES):
        x4 = x[NL * k:NL * (k + 1)].reshape(2, 2, C, HW)
        xp = np.ascontiguousarray(
            x4.transpose(0, 2, 1, 3).reshape(128, WIDE)).astype(bf16)
        in_maps.append({"xp": xp, "xst": xst_f8, "eye": eye})
    return in_maps


def _unshard_output(results):
    y = np.empty((N, C, H, W), dtype=np.float32)
    for k in range(CORES):
        o = np.asarray(results[k]["out"]).astype(np.float32)
        o = o.reshape(16, 128, 2048).transpose(1, 0, 2).reshape(128, WIDE)
        o = o.reshape(2, C, 2, HW)
        y[NL * k:NL * (k + 1)] = (
            o.transpose(0, 2, 1, 3).reshape(NL, C, H, W))
    return y


def kernel(x):
    from concourse.bass_utils import run_bass_kernel_spmd

    if "nc" not in _cached:
        _cached["nc"] = _build_nc()
    nc = _cached["nc"]

    in_maps = _shard_inputs(np.asarray(x))
    res = run_bass_kernel_spmd(nc, in_maps, core_ids=list(range(CORES)))
    _cached["last_results"] = res
    return _unshard_output(res.results)
